# revision 17
# baseline (speedup 1.0000x reference)
"""Local 9x9 correlation (cost volume) kernel for Trainium2.

out[b, di*9+dj, h, w] = (1/C) * sum_c x1[b,c,h,w] * x2pad[b,c,h+di-4,w+dj-4]

Strategy: batch-parallel across 8 NeuronCores (1 sample each). On-core, the
output is tiled into 6 row-bands x 16 col-groups; per (band, group) the
stationary operand is the 16x8 block of x1 positions (all 128 PE columns,
host-blocked so the slice is one contiguous free dim) and the moving operand
is the 24x16 zero-padded x2 window around it (N=384 columns), accumulated
over C=256 in fp32 PSUM with two fp16 matmuls. The [128, 384] band is cast to
fp16 and dumped to DRAM as two 64-partition halves trimmed to the 256 columns
each half's outputs can touch (512B descriptors); the host extracts the 81
displacement diagonals and applies the 1/C scale.

Inputs are converted to fp16 on the host (x1 additionally blocked per
band/group, x2 zero-padded in W) so the device reads the minimum bytes.
"""

import numpy as np

B, C, H, W = 8, 256, 96, 128
R = 4                  # correlation radius
D = 2 * R + 1          # 9 displacements per axis
G = 16                 # output rows per band
S = 8                  # output cols per group
NB = H // G            # 6 bands
NG = W // S            # 16 groups per band
WR = G + 2 * R         # 24 moving rows per band
WC = S + 2 * R         # 16 moving cols per group
N = WR * WC            # 384 moving columns per matmul
PADW = W + 2 * R       # 136 (host-padded)
PADH = H + 2 * R       # 104 (device memset rows)
QW = 256               # dumped columns per 64-partition half-band

_compiled = None
last_results = None  # BassKernelResults of the most recent run (for profiling)


def _build(reps: int = 1):
    import contextlib

    import concourse.bass as bass  # noqa: F401
    import concourse.tile as tile
    from concourse import bacc, mybir

    nc = bacc.Bacc(
        "TRN2", target_bir_lowering=False, debug=False, num_devices=8
    )
    x1 = nc.dram_tensor(
        "x1", [128, 2, NB, NG, G * S], mybir.dt.float16, kind="ExternalInput"
    ).ap()
    x2 = nc.dram_tensor(
        "x2", [128, 2, H, W], mybir.dt.float16, kind="ExternalInput"
    ).ap()
    dump = nc.dram_tensor(
        "dump", [NB, 2, 64, NG, QW], mybir.dt.float16, kind="ExternalOutput"
    ).ap()

    NCH = 12  # 8-row x2 load chunks

    with tile.TileContext(nc) as tc:
        with (
            tc.tile_pool(name="x1p", bufs=2) as x1p,
            tc.tile_pool(name="x2p", bufs=2) as x2p,
            tc.tile_pool(name="stg", bufs=6) as stg,
            tc.tile_pool(name="ps", bufs=8, space="PSUM") as psp,
            tc.For_i(0, reps, 1) if reps > 1 else contextlib.nullcontext(),
        ):
            # x1 in two half tiles (bands 0-2 / 3-5) so the next rep's x1
            # loads only wait on that half's readers
            x1h0 = x1p.tile(
                [128, 2, NB // 2, NG, G * S], mybir.dt.float16, tag="x1h"
            )
            x1h1 = x1p.tile(
                [128, 2, NB // 2, NG, G * S], mybir.dt.float16, tag="x1h"
            )
            x1sb = [x1h0, x1h1]
            # x2 kept flat per partition: elem(padded row r, col c) lives at
            # 4 + r*W + c. Window reads for edge groups run past row ends
            # into the neighbouring row (or the memset guards); the affected
            # band columns only feed outputs whose displacement leaves the
            # image, which the host zeroes exactly.
            FL = 4 + PADH * W + 116  # 116 tail so the 24x128 view slices stay in range
            x2sb = x2p.tile([128, 2, FL], mybir.dt.float16)
            nc.vector.memset(x2sb[:, :, 0 : 4 + R * W], 0.0)
            nc.vector.memset(
                x2sb[:, :, 4 + (R + H) * W : 4 + PADH * W + 4], 0.0
            )

            # All loads and dumps issue from SP (sync) in one in-order
            # stream so load and dump transfers interleave on the DMA
            # engines instead of dumps queueing behind every load.
            def load_x2(k):
                if k < NCH:
                    a = 4 + (R + 8 * k) * W
                    nc.sync.dma_start(
                        out=x2sb[:, :, a : a + 8 * W].rearrange(
                            "p c (r w) -> p c r w", r=8, w=W
                        ),
                        in_=x2[:, :, 8 * k : 8 * (k + 1), :],
                    )

            def load_x1(hb):
                if hb < NB:
                    nc.sync.dma_start(
                        out=x1sb[hb // 3][:, :, hb % 3, :, :],
                        in_=x1[:, :, hb, :, :],
                    )

            def load_band(hb):
                load_x2(2 * hb + 1)
                load_x2(2 * hb + 2)
                load_x1(hb)

            # prologue: inputs for bands 0..2
            load_x2(0)
            for hb in range(3):
                load_band(hb)

            HG = NG // 2  # groups per half-band stage
            for hb in range(NB):
                for half in range(2):
                    stage = stg.tile([128, HG, N], mybir.dt.float16)
                    for gl in range(HG):
                        g = half * HG + gl
                        ps = psp.tile([128, N], mybir.dt.float32)
                        off = hb * G * W + g * S  # 4 + (hb*G)*W + (g*S - R)
                        for cc in range(2):
                            nc.tensor.matmul(
                                out=ps[:, :],
                                lhsT=x1sb[hb // 3][:, cc, hb % 3, g, :],
                                rhs=x2sb[
                                    :, cc, off : off + WR * W
                                ].rearrange("p (r w) -> p r w", r=WR, w=W)[
                                    :, :, 0:WC
                                ],
                                start=(cc == 0),
                                stop=(cc == 1),
                            )
                        if gl % 2 == 0:
                            nc.vector.tensor_scalar_mul(
                                stage[:, gl, :], ps[:, :], 1.0
                            )
                        else:
                            nc.scalar.mul(stage[:, gl, :], ps[:, :], 1.0)
                    for q in range(2):
                        nc.sync.dma_start(
                            out=dump[
                                hb, q, :, half * HG : (half + 1) * HG, :
                            ],
                            in_=stage[
                                64 * q : 64 * (q + 1),
                                :,
                                128 * q : 128 * q + QW,
                            ],
                        )
                # after band hb's dumps are issued, queue band hb+3 loads
                load_band(hb + 3)

    nc.compile()
    return nc


def _prep_inputs(x1: np.ndarray, x2: np.ndarray):
    """fp32 [C,H,W] per-sample -> fp16 channel-split device layouts."""
    x1h = (
        x1.astype(np.float16)
        .reshape(2, 128, NB, G, NG, S)
        .transpose(1, 0, 2, 4, 3, 5)   # [128, 2, hb, g, rl, wm]
        .reshape(128, 2, NB, NG, G * S)
        .copy()
    )
    x2h = np.ascontiguousarray(
        x2.astype(np.float16).reshape(2, 128, H, W).transpose(1, 0, 2, 3)
    )
    return x1h, x2h


# host deskew gather indices (see check_math.py)
_s = np.arange(G // 2)[:, None]
_wm = np.arange(S)[None, :]
_IDXP = S * _s + _wm                                             # (8,8)
_di = np.arange(D)[:, None, None, None]
_dj = np.arange(D)[None, :, None, None]
_IDXJ = (_s[None, None] + _di) * WC + _wm[None, None] + _dj      # (9,9,8,8)


# out[:, dj, :, w] crosses the W edge iff w+dj < R or w+dj-R >= W
_WEDGE = [
    np.flatnonzero((np.arange(W) + dj < R) | (np.arange(W) + dj - R >= W))
    for dj in range(D)
]


def _deskew(dump_b: np.ndarray) -> np.ndarray:
    """[NB, 2, 64, NG, QW] fp16 half-band dump -> [81, H, W] fp32."""
    d = np.asarray(dump_b).astype(np.float32)
    g1 = d[:, :, _IDXP[None, None], :, _IDXJ]  # (9,9,8,8,NB,2,NG)
    out = g1.transpose(0, 1, 4, 5, 2, 6, 3).reshape(D, D, H, W)
    for dj in range(D):
        out[:, dj, :, _WEDGE[dj]] = 0.0
    return out.reshape(D * D, H, W) * np.float32(1.0 / C)


def kernel(x1: np.ndarray, x2: np.ndarray) -> np.ndarray:
    global _compiled, last_results
    import os

    os.environ["BASS_NEVER_TRACE"] = "1"
    from concourse.bass_utils import run_bass_kernel_spmd

    x1 = np.ascontiguousarray(np.asarray(x1), dtype=np.float32)
    x2 = np.ascontiguousarray(np.asarray(x2), dtype=np.float32)
    assert x1.shape == (B, C, H, W) and x2.shape == (B, C, H, W)

    if _compiled is None:
        _compiled = _build()
    nc = _compiled

    in_maps = []
    for b in range(B):
        x1h, x2h = _prep_inputs(x1[b], x2[b])
        in_maps.append({"x1": x1h, "x2": x2h})
    res = run_bass_kernel_spmd(nc, in_maps, core_ids=list(range(B)))
    last_results = res

    return np.stack([_deskew(res.results[b]["dump"]) for b in range(B)], axis=0)


def _timed_run(nc, x1, x2, iters):
    import time

    import jax
    from jax.experimental.shard_map import shard_map
    from jax.sharding import Mesh, PartitionSpec

    from concourse import bass2jax, mybir

    bass2jax.install_neuronx_cc_hook()

    partition_name = (
        nc.partition_id_tensor.name if nc.partition_id_tensor else None
    )
    in_names, out_names, out_avals, zeros = [], [], [], []
    for alloc in nc.m.functions[0].allocations:
        if not isinstance(alloc, mybir.MemoryLocationSet):
            continue
        name = alloc.memorylocations[0].name
        if alloc.kind == "ExternalInput":
            if name != partition_name:
                in_names.append(name)
        elif alloc.kind == "ExternalOutput":
            shape = tuple(alloc.tensor_shape)
            dtype = mybir.dt.np(alloc.dtype)
            out_names.append(name)
            out_avals.append(jax.core.ShapedArray(shape, dtype))
            zeros.append(np.zeros(shape, dtype))
    n_params = len(in_names)
    all_names = in_names + out_names
    if partition_name is not None:
        all_names = all_names + [partition_name]

    def _body(*args):
        operands = list(args)
        if partition_name is not None:
            operands.append(bass2jax.partition_id_tensor())
        return tuple(
            bass2jax._bass_exec_p.bind(
                *operands,
                out_avals=tuple(out_avals),
                in_names=tuple(all_names),
                out_names=tuple(out_names),
                lowering_input_output_aliases=(),
                sim_require_finite=True,
                sim_require_nnan=True,
                nc=nc,
            )
        )

    devices = jax.devices()[:B]
    mesh = Mesh(np.asarray(devices), ("core",))
    specs = (PartitionSpec("core"),) * (n_params + len(out_names))

    fn = jax.jit(
        shard_map(
            _body,
            mesh=mesh,
            in_specs=specs,
            out_specs=(PartitionSpec("core"),) * len(out_names),
            check_rep=False,
        ),
        keep_unused=True,
    )

    per = {"x1": [], "x2": []}
    for b in range(B):
        x1h, x2h = _prep_inputs(x1[b], x2[b])
        per["x1"].append(x1h)
        per["x2"].append(x2h)
    concat_in = [np.concatenate(per[n], axis=0) for n in in_names]
    concat_zero = [
        np.zeros((B * z.shape[0], *z.shape[1:]), z.dtype) for z in zeros
    ]
    sharding = jax.sharding.NamedSharding(mesh, PartitionSpec("core"))
    dev_args = [jax.device_put(a, sharding) for a in concat_in + concat_zero]

    outs = fn(*dev_args)
    jax.block_until_ready(outs)
    ts = []
    for _ in range(iters):
        t0 = time.perf_counter()
        outs = fn(*dev_args)
        jax.block_until_ready(outs)
        ts.append(time.perf_counter() - t0)
    ts.sort()
    return ts


REPS_LONG = 65


def benchmark(x1: np.ndarray, x2: np.ndarray, iters: int = 10):
    """Per-execution device time via reps-loop slope: two NEFFs (reps=1 and
    reps=REPS_LONG with an on-device For_i around the body); the wall-clock
    difference divided by (REPS_LONG-1) cancels the axon dispatch overhead."""
    nc1 = _build(1)
    t1 = _timed_run(nc1, x1, x2, iters)
    ncN = _build(REPS_LONG)
    tN = _timed_run(ncN, x1, x2, iters)
    per_exec = (tN[0] - t1[0]) / (REPS_LONG - 1)
    return per_exec, t1, tN
